# revision 10
# baseline (speedup 1.0000x reference)
"""Trainium2 Bass kernel for nn_BWCaster_86337432584570 (embedding_lookup), v3.

sigma[n,j] = relu( sum_p sum_c bilinear(plane_p[j])[c] * linear(line_p[j])[c] )

v3 design: 3 fused 256B gather rows per (point, joint) — each plane's row
carries a line's 2 exact taps in its padding (the line coordinate equals an
exact row/col index of some plane):
  row0 (y=|g1|, x=|g0|): P0[c,2y,2x] (64) | L2[c, x:x+2] (32) | pad
  row1 (y=|g2|, x=|g0|): P1[c,2y,2x]      | L0[c, y:y+2]      | pad
  row2 (y=|g2|, x=|g1|): P2[c,2y,2x]      | L1[c, x:x+2]      | pad
Gathers run on 4 SWDGE queues (6 calls/iter, 12 slots per queue) — the
per-queue descriptor ring is the bottleneck (~7-9 ns/desc/queue).
"""
import sys
import numpy as np
import ml_dtypes

sys.path.insert(0, "/opt/trn_rl_repo")

import concourse.bass as bass
import concourse.bacc as bacc
import concourse.mybir as mybir
from concourse.bass_utils import run_bass_kernel_spmd
from concourse.library_config import mlp

# ---------------- problem constants (hardcoded) ----------------
N_TOTAL, J, C, G = 262144, 24, 16, 128
N_CORES = 8
NPTS = N_TOTAL // N_CORES          # 32768 points per core
SB = 2048                          # points per superblock
NSB = NPTS // SB                   # 16
NB = SB // 128                     # 16 sub-blocks of 128 points
NSLOT = 3 * NB                     # 48 gather slots per (j, superblock)
NIDX = NSLOT * 128                 # 6144 indices per (j, superblock)
DEPTH = 5                          # pipeline depth (gather buffers)
NROWS = 127 * 127                  # 16129 rows per (j, p) table
WLEN = 3 * NB * 4 + 3 * NB * 2     # 288 weights per partition-point
# gather sub-calls: (p, slot_lo, slot_hi, queue)
GCALLS = [(0, 0, 12, 0), (0, 12, 16, 1),
          (1, 0, 8, 1), (1, 8, 16, 2),
          (2, 0, 4, 2), (2, 4, 16, 3)]
BF16 = mybir.dt.bfloat16
F32 = mybir.dt.float32
I16 = mybir.dt.int16

_CACHE = {}


# ---------------- host-side prep ----------------
def _build_tables(planes, lines):
    """tab [3, J, NROWS, 128] bf16; row (y,x) of table p:
    [16c x (2y,2x)] plane corners | [16c x 2] line taps | 32 pad."""
    tab = np.zeros((3, J, NROWS, 128), dtype=np.float32)
    for p in range(3):
        sw = np.lib.stride_tricks.sliding_window_view(planes[p], (2, 2), axis=(2, 3))
        # [J, C, 127, 127, 2, 2] -> [J, y, x, C, 2, 2]
        tab[p, :, :, 0:64] = sw.transpose(0, 2, 3, 1, 4, 5).reshape(J, NROWS, 64)
    # line tails (2 exact taps, broadcast over the unused row coordinate)
    l2 = np.lib.stride_tricks.sliding_window_view(lines[2], 2, axis=2)  # [J,C,127,2]
    t0 = np.broadcast_to(l2.transpose(0, 2, 1, 3)[:, None, :, :, :],
                         (J, 127, 127, C, 2))                  # bcast over y
    tab[0, :, :, 64:96] = t0.reshape(J, NROWS, 32)
    l0 = np.lib.stride_tricks.sliding_window_view(lines[0], 2, axis=2)
    t1 = np.broadcast_to(l0.transpose(0, 2, 1, 3)[:, :, None, :, :],
                         (J, 127, 127, C, 2))                  # bcast over x
    tab[1, :, :, 64:96] = t1.reshape(J, NROWS, 32)
    l1 = np.lib.stride_tricks.sliding_window_view(lines[1], 2, axis=2)
    t2 = np.broadcast_to(l1.transpose(0, 2, 1, 3)[:, None, :, :, :],
                         (J, 127, 127, C, 2))                  # bcast over y
    tab[2, :, :, 64:96] = t2.reshape(J, NROWS, 32)
    return tab.astype(ml_dtypes.bfloat16)


def _coords_weights(xyz, transforms):
    """Returns idx3 [N,J,3] int16, wp [N,J,3,4] f32, wl [N,J,3,2] f32.
    wl is in ROW order (row0=L2, row1=L0, row2=L1)."""
    N = xyz.shape[0]
    xyzh = np.concatenate([xyz, np.ones((N, 1), np.float32)], axis=1)
    pts = np.einsum('jab,nb->nja', transforms[:, :3, :].astype(np.float32), xyzh)
    coord = (pts * np.float32(2.0 / 3.0) + np.float32(1.0)) * np.float32(0.5 * (G - 1))
    c0 = np.floor(coord).astype(np.int32)          # [N,J,3] per-axis cell
    fr = (coord - c0).astype(np.float32)
    c0c = np.clip(c0, 0, 126)                      # safety; margin makes this a no-op

    # (y_axis, x_axis) per plane row: row0=(1,0), row1=(2,0), row2=(2,1)
    YX = [(1, 0), (2, 0), (2, 1)]
    idx3 = np.empty((N, J, 3), np.int16)
    wp = np.empty((N, J, 3, 4), np.float32)
    wl = np.empty((N, J, 3, 2), np.float32)
    for p, (ya, xa) in enumerate(YX):
        y0, x0 = c0c[:, :, ya], c0c[:, :, xa]
        fy, fx = fr[:, :, ya], fr[:, :, xa]
        idx3[:, :, p] = (y0 * 127 + x0).astype(np.int16)
        wy0, wx0 = 1.0 - fy, 1.0 - fx
        wp[:, :, p, 0] = wy0 * wx0
        wp[:, :, p, 1] = wy0 * fx
        wp[:, :, p, 2] = fy * wx0
        wp[:, :, p, 3] = fy * fx
    # line taps: row0 tail = L2 @ g0 (axis 0), row1 = L0 @ g2, row2 = L1 @ g1
    for p, ax in enumerate([0, 2, 1]):
        f = fr[:, :, ax]
        wl[:, :, p, 0] = 1.0 - f
        wl[:, :, p, 1] = f
    return idx3, wp, wl


def _pack_core(idx3, wp, wl):
    """idx3 [NPTS,J,3] -> idx dram [J,NSB,128,NIDX//16] int16 (wrapped+replicated)
    wp/wl -> w dram [J,NSB,128,WLEN] bf16: [3p,16nb,4 | 3row,16nb,2]."""
    u = idx3.reshape(NSB, NB, 128, J, 3)
    arr = u.transpose(3, 0, 4, 1, 2).reshape(J, NSB, NSLOT * 128)
    wrapped = arr.reshape(J, NSB, NIDX // 16, 16).transpose(0, 1, 3, 2)
    idx_dram = np.broadcast_to(wrapped[:, :, None, :, :], (J, NSB, 8, 16, NIDX // 16))
    idx_dram = np.ascontiguousarray(idx_dram).reshape(J, NSB, 128, NIDX // 16)

    a = wp.reshape(NSB, NB, 128, J, 3, 4).transpose(3, 0, 2, 4, 1, 5)  # [j,sb,np,p,nb,4]
    b = wl.reshape(NSB, NB, 128, J, 3, 2).transpose(3, 0, 2, 4, 1, 5)  # [j,sb,np,r,nb,2]
    w_dram = np.concatenate([
        np.ascontiguousarray(a).reshape(J, NSB, 128, 192),
        np.ascontiguousarray(b).reshape(J, NSB, 128, 96),
    ], axis=3).astype(ml_dtypes.bfloat16)
    return idx_dram, w_dram


# ---------------- device kernel ----------------
def _build_bass(nit_lim=None):
    """nit_lim > 384 wraps (for slope timing); real workload is NIT=384."""
    nc = bacc.Bacc("TRN2", num_swdge_queues=4)
    tab = nc.dram_tensor("tab", [3, J, NROWS, 128], BF16, kind="ExternalInput")
    idx = nc.dram_tensor("idx", [J, NSB, 128, NIDX // 16], I16, kind="ExternalInput")
    w8 = nc.dram_tensor("w8", [J, NSB, 128, WLEN], BF16, kind="ExternalInput")
    # device-native layout [np, sb, nb, j]; host transposes to [NPTS, J]
    out = nc.dram_tensor("out", [128, NSB, NB, J], F32, kind="ExternalOutput")

    NIT = J * NSB  # 384 iterations, j outer / sb inner
    if nit_lim is not None:
        NIT = nit_lim
    D = DEPTH
    NCALL = len(GCALLS)

    from contextlib import ExitStack
    with ExitStack() as ctx:
        dst = ctx.enter_context(nc.sbuf_tensor("dst", [128, D, NSLOT, 128], BF16))
        idxs = ctx.enter_context(nc.sbuf_tensor("idxs", [128, D, NIDX // 16], I16))
        w8t = ctx.enter_context(nc.sbuf_tensor("w8t", [128, D, WLEN], BF16))
        wprod = ctx.enter_context(nc.sbuf_tensor("wprod", [128, 3072], BF16))
        t1 = ctx.enter_context(nc.sbuf_tensor("t1", [128, 1536], BF16))
        pf = ctx.enter_context(nc.sbuf_tensor("pf", [128, 768], F32))
        lft = ctx.enter_context(nc.sbuf_tensor("lft", [128, 1536], BF16))
        lf = ctx.enter_context(nc.sbuf_tensor("lf", [128, 768], F32))
        prod = ctx.enter_context(nc.sbuf_tensor("prod", [128, NB, 3, 16], F32))
        outt = ctx.enter_context(nc.sbuf_tensor("outt", [128, NSB, NB, J], F32))
        s_gat = [ctx.enter_context(nc.semaphore(f"s_gat{i}")) for i in range(D)]
        s_idx = [ctx.enter_context(nc.semaphore(f"s_idx{i}")) for i in range(D)]
        s_w8 = [ctx.enter_context(nc.semaphore(f"s_w8{i}")) for i in range(D)]
        s_cmb = ctx.enter_context(nc.semaphore("s_cmb"))
        s_relu = ctx.enter_context(nc.semaphore("s_relu"))
        s_out = ctx.enter_context(nc.semaphore("s_out"))
        s_v = ctx.enter_context(nc.semaphore("s_v"))
        block = ctx.enter_context(nc.Block())

        @block.gpsimd
        def _(gpsimd):
            gpsimd.load_library(mlp)
            for it in range(NIT):
                e = it % 384
                j = e // NSB
                b = it % D
                if it >= D:
                    # dst[b] free once compute(it-D) done
                    gpsimd.wait_ge(s_cmb, it - (D - 1))
                gpsimd.wait_ge(s_idx[b], 16 * (it // D + 1))
                for (p, lo, hi, q) in GCALLS:
                    s0 = p * NB + lo
                    s1 = p * NB + hi
                    n = (hi - lo) * 128
                    gpsimd.dma_gather(
                        dst[:, b, s0:s1, :], tab[p, j],
                        idxs[:, b, s0 * 8:s1 * 8],
                        n, n, 128, single_packet=False, queue_num=q,
                    ).then_inc(s_gat[b], 16)

        @block.sync
        def _(sync):
            for it in range(NIT):
                e = it % 384
                j, sb = e // NSB, e % NSB
                b = it % D
                if it >= D:
                    # w8t[b] free once compute(it-D) done
                    sync.wait_ge(s_cmb, it - (D - 1))
                    # idxs[b] consumed once gather(it-D) completed
                    sync.wait_ge(s_gat[b], 16 * NCALL * (it // D))
                sync.dma_start(idxs[:, b, :], idx[j, sb]).then_inc(s_idx[b], 16)
                sync.dma_start(w8t[:, b, :], w8[j, sb]).then_inc(s_w8[b], 16)
            # final output DMA after relu (contiguous, same layout)
            sync.wait_ge(s_relu, 1)
            sync.dma_start(out[:], outt[:]).then_inc(s_out, 16)
            sync.wait_ge(s_out, 16)

        @block.vector
        def _(vector):
            sv = 0

            def emit(inst):
                nonlocal sv
                sv += 1
                inst.then_inc(s_v, 1)

            def barrier():
                vector.wait_ge(s_v, sv)

            emit(vector.memset(outt[:].rearrange("P a b c -> P (a b c)"), 0.0))
            barrier()
            for it in range(NIT):
                e = it % 384
                j, sb = e // NSB, e % NSB
                b = it % D
                vector.wait_ge(s_gat[b], 16 * NCALL * (it // D + 1))
                vector.wait_ge(s_w8[b], 16 * (it // D + 1))
                # plane corners [3p,16nb | 16c,4yx] * wp [3p,16nb,4] bcast c
                in0 = dst[:, b, :, 0:64].rearrange(
                    "P (p nb) (c r) -> P p nb c r", p=3, c=16)
                in1 = w8t[:, b, 0:192].rearrange(
                    "P (p nb r) -> P p nb r", p=3, nb=NB
                ).unsqueeze(3).broadcast_to([128, 3, NB, 16, 4])
                wv = wprod[:].rearrange(
                    "P (p nb c r) -> P p nb c r", p=3, nb=NB, c=16)
                emit(vector.tensor_tensor(wv, in0, in1, mybir.AluOpType.mult))
                # line taps [3row,16nb | 16c,2] * wl [3row,16nb,2] bcast c
                li0 = dst[:, b, :, 64:96].rearrange(
                    "P (p nb) (c r) -> P p nb c r", p=3, c=16)
                li1 = w8t[:, b, 192:288].rearrange(
                    "P (p nb r) -> P p nb r", p=3, nb=NB
                ).unsqueeze(3).broadcast_to([128, 3, NB, 16, 2])
                lv = lft[:].rearrange(
                    "P (p nb c r) -> P p nb c r", p=3, nb=NB, c=16)
                emit(vector.tensor_tensor(lv, li0, li1, mybir.AluOpType.mult))
                barrier()
                # plane tree 4 -> 2 -> 1 ; line tree 2 -> 1
                w3 = wprod[:].rearrange("P (m r) -> P m r", r=4)
                t1v = t1[:].rearrange("P (m r) -> P m r", r=2)
                emit(vector.tensor_tensor(t1v, w3[:, :, 0:2], w3[:, :, 2:4],
                                          mybir.AluOpType.add))
                l3 = lft[:].rearrange("P (m r) -> P m r", r=2)
                emit(vector.tensor_tensor(lf[:], l3[:, :, 0], l3[:, :, 1],
                                          mybir.AluOpType.add))
                barrier()
                emit(vector.tensor_tensor(pf[:], t1v[:, :, 0], t1v[:, :, 1],
                                          mybir.AluOpType.add))
                barrier()
                # prod[nb, p, c] = pf[p, nb, c] * lf[rowof(p), nb, c]
                pfv = pf[:].rearrange("P (p nb c) -> P p nb c", p=3, c=16)
                lfv = lf[:].rearrange("P (p nb c) -> P p nb c", p=3, c=16)
                prodv = prod[:].rearrange("P nb p c -> P p nb c")
                emit(vector.tensor_tensor(
                    prodv[:, 0:2], pfv[:, 0:2], lfv[:, 1:3],
                    mybir.AluOpType.mult))
                emit(vector.tensor_tensor(
                    prodv[:, 2], pfv[:, 2], lfv[:, 0], mybir.AluOpType.mult))
                barrier()
                vector.tensor_reduce(
                    outt[:, sb, :, j],
                    prod[:].rearrange("P nb p c -> P nb (p c)"),
                    mybir.AxisListType.X, mybir.AluOpType.add,
                ).then_inc(s_cmb, 1)
            vector.wait_ge(s_cmb, NIT)
            of = outt[:].rearrange("P a b c -> P (a b c)")
            vector.tensor_scalar_max(of, of, 0.0).then_inc(s_relu, 1)

    nc.compile()
    return nc


# ---------------- entry point ----------------
def prepare_in_maps(inputs):
    planes = [np.asarray(inputs[f"plane{i}"]) for i in range(3)]
    lines = [np.asarray(inputs[f"line{i}"]) for i in range(3)]
    tab = _build_tables(planes, lines)
    idx3, wp, wl = _coords_weights(
        np.asarray(inputs["xyz"]), np.asarray(inputs["transforms"]))
    in_maps = []
    for k in range(N_CORES):
        s = slice(k * NPTS, (k + 1) * NPTS)
        idx_d, w_d = _pack_core(idx3[s], wp[s], wl[s])
        in_maps.append({"tab": tab, "idx": idx_d, "w8": w_d})
    return in_maps


def kernel(xyz, transforms, plane0, plane1, plane2, line0, line1, line2):
    in_maps = prepare_in_maps(dict(
        xyz=xyz, transforms=transforms, plane0=plane0, plane1=plane1,
        plane2=plane2, line0=line0, line1=line1, line2=line2))

    if "nc" not in _CACHE:
        _CACHE["nc"] = _build_bass()
    nc = _CACHE["nc"]

    _CACHE["in_maps"] = in_maps
    res = run_bass_kernel_spmd(nc, in_maps, core_ids=list(range(N_CORES)))
    outs = []
    for r in res.results:
        o = np.asarray(r["out"]).reshape(128, NSB, NB, J)
        outs.append(o.transpose(1, 2, 0, 3).reshape(NPTS, J))
    return np.concatenate(outs, axis=0).astype(np.float32)


if __name__ == "__main__":
    rng = np.random.default_rng(0)
    xyz = (rng.random((N_TOTAL, 3), np.float32) * 2 - 1).astype(np.float32)
    tr = (np.eye(4, dtype=np.float32)[None]
          + 0.05 * rng.standard_normal((J, 4, 4)).astype(np.float32))
    pl = [(0.032 * rng.standard_normal((J, C, G, G))).astype(np.float32) for _ in range(3)]
    ln = [(0.032 * rng.standard_normal((J, C, G))).astype(np.float32) for _ in range(3)]
    o = kernel(xyz, tr, pl[0], pl[1], pl[2], ln[0], ln[1], ln[2])
    print(o.shape, o.dtype, float(o.max()))


# revision 11
# speedup vs baseline: 1.1767x; 1.1767x over previous
"""Trainium2 Bass kernel for nn_BWCaster_86337432584570 (embedding_lookup), v3.

sigma[n,j] = relu( sum_p sum_c bilinear(plane_p[j])[c] * linear(line_p[j])[c] )

v3 design: 3 fused 256B gather rows per (point, joint) — each plane's row
carries a line's 2 exact taps in its padding (the line coordinate equals an
exact row/col index of some plane):
  row0 (y=|g1|, x=|g0|): P0[c,2y,2x] (64) | L2[c, x:x+2] (32) | pad
  row1 (y=|g2|, x=|g0|): P1[c,2y,2x]      | L0[c, y:y+2]      | pad
  row2 (y=|g2|, x=|g1|): P2[c,2y,2x]      | L1[c, x:x+2]      | pad
Gathers run on 4 SWDGE queues (6 calls/iter, 12 slots per queue) — the
per-queue descriptor ring is the bottleneck (~7-9 ns/desc/queue).
"""
import sys
import numpy as np
import ml_dtypes

sys.path.insert(0, "/opt/trn_rl_repo")

import concourse.bass as bass
import concourse.bacc as bacc
import concourse.mybir as mybir
from concourse.bass_utils import run_bass_kernel_spmd
from concourse.library_config import mlp

# ---------------- problem constants (hardcoded) ----------------
N_TOTAL, J, C, G = 262144, 24, 16, 128
N_CORES = 8
NPTS = N_TOTAL // N_CORES          # 32768 points per core
SB = 2048                          # points per superblock
NSB = NPTS // SB                   # 16
NB = SB // 128                     # 16 sub-blocks of 128 points
NSLOT = 3 * NB                     # 48 gather slots per (j, superblock)
NIDX = NSLOT * 128                 # 6144 indices per (j, superblock)
DEPTH = 4                          # pipeline depth (gather buffers)
NROWS = 127 * 127                  # 16129 rows per (j, p) table
WLEN = 3 * NB * 4 + 3 * NB * 2     # 288 weights per partition-point
# gather sub-calls: (p, slot_lo, slot_hi, queue)
GCALLS = [(0, 0, 12, 0), (0, 12, 16, 1),
          (1, 0, 8, 1), (1, 8, 16, 2),
          (2, 0, 4, 2), (2, 4, 16, 3)]
BF16 = mybir.dt.bfloat16
F32 = mybir.dt.float32
I16 = mybir.dt.int16

_CACHE = {}


# ---------------- host-side prep ----------------
def _build_tables(planes, lines):
    """tab [3, J, NROWS, 128] bf16; row (y,x) of table p:
    [16c x (2y,2x)] plane corners | [16c x 2] line taps | 32 pad."""
    tab = np.zeros((3, J, NROWS, 128), dtype=np.float32)
    for p in range(3):
        sw = np.lib.stride_tricks.sliding_window_view(planes[p], (2, 2), axis=(2, 3))
        # [J, C, 127, 127, 2, 2] -> [J, y, x, C, 2, 2]
        tab[p, :, :, 0:64] = sw.transpose(0, 2, 3, 1, 4, 5).reshape(J, NROWS, 64)
    # line tails (2 exact taps, broadcast over the unused row coordinate)
    l2 = np.lib.stride_tricks.sliding_window_view(lines[2], 2, axis=2)  # [J,C,127,2]
    t0 = np.broadcast_to(l2.transpose(0, 2, 1, 3)[:, None, :, :, :],
                         (J, 127, 127, C, 2))                  # bcast over y
    tab[0, :, :, 64:96] = t0.reshape(J, NROWS, 32)
    l0 = np.lib.stride_tricks.sliding_window_view(lines[0], 2, axis=2)
    t1 = np.broadcast_to(l0.transpose(0, 2, 1, 3)[:, :, None, :, :],
                         (J, 127, 127, C, 2))                  # bcast over x
    tab[1, :, :, 64:96] = t1.reshape(J, NROWS, 32)
    l1 = np.lib.stride_tricks.sliding_window_view(lines[1], 2, axis=2)
    t2 = np.broadcast_to(l1.transpose(0, 2, 1, 3)[:, None, :, :, :],
                         (J, 127, 127, C, 2))                  # bcast over y
    tab[2, :, :, 64:96] = t2.reshape(J, NROWS, 32)
    return tab.astype(ml_dtypes.bfloat16)


def _coords_weights(xyz, transforms):
    """Returns idx3 [N,J,3] int16, wp [N,J,3,4] f32, wl [N,J,3,2] f32.
    wl is in ROW order (row0=L2, row1=L0, row2=L1)."""
    N = xyz.shape[0]
    xyzh = np.concatenate([xyz, np.ones((N, 1), np.float32)], axis=1)
    pts = np.einsum('jab,nb->nja', transforms[:, :3, :].astype(np.float32), xyzh)
    coord = (pts * np.float32(2.0 / 3.0) + np.float32(1.0)) * np.float32(0.5 * (G - 1))
    c0 = np.floor(coord).astype(np.int32)          # [N,J,3] per-axis cell
    fr = (coord - c0).astype(np.float32)
    c0c = np.clip(c0, 0, 126)                      # safety; margin makes this a no-op

    # (y_axis, x_axis) per plane row: row0=(1,0), row1=(2,0), row2=(2,1)
    YX = [(1, 0), (2, 0), (2, 1)]
    idx3 = np.empty((N, J, 3), np.int16)
    wp = np.empty((N, J, 3, 4), np.float32)
    wl = np.empty((N, J, 3, 2), np.float32)
    for p, (ya, xa) in enumerate(YX):
        y0, x0 = c0c[:, :, ya], c0c[:, :, xa]
        fy, fx = fr[:, :, ya], fr[:, :, xa]
        idx3[:, :, p] = (y0 * 127 + x0).astype(np.int16)
        wy0, wx0 = 1.0 - fy, 1.0 - fx
        wp[:, :, p, 0] = wy0 * wx0
        wp[:, :, p, 1] = wy0 * fx
        wp[:, :, p, 2] = fy * wx0
        wp[:, :, p, 3] = fy * fx
    # line taps: row0 tail = L2 @ g0 (axis 0), row1 = L0 @ g2, row2 = L1 @ g1
    for p, ax in enumerate([0, 2, 1]):
        f = fr[:, :, ax]
        wl[:, :, p, 0] = 1.0 - f
        wl[:, :, p, 1] = f
    return idx3, wp, wl


def _pack_core(idx3, wp, wl):
    """idx3 [NPTS,J,3] -> idx dram [J,NSB,128,NIDX//16] int16 (wrapped+replicated)
    wp/wl -> w dram [J,NSB,128,WLEN] bf16: [3p,16nb,4 | 3row,16nb,2]."""
    u = idx3.reshape(NSB, NB, 128, J, 3)
    arr = u.transpose(3, 0, 4, 1, 2).reshape(J, NSB, NSLOT * 128)
    wrapped = arr.reshape(J, NSB, NIDX // 16, 16).transpose(0, 1, 3, 2)
    idx_dram = np.broadcast_to(wrapped[:, :, None, :, :], (J, NSB, 8, 16, NIDX // 16))
    idx_dram = np.ascontiguousarray(idx_dram).reshape(J, NSB, 128, NIDX // 16)

    a = wp.reshape(NSB, NB, 128, J, 3, 4).transpose(3, 0, 2, 4, 1, 5)  # [j,sb,np,p,nb,4]
    b = wl.reshape(NSB, NB, 128, J, 3, 2).transpose(3, 0, 2, 4, 1, 5)  # [j,sb,np,r,nb,2]
    w_dram = np.concatenate([
        np.ascontiguousarray(a).reshape(J, NSB, 128, 192),
        np.ascontiguousarray(b).reshape(J, NSB, 128, 96),
    ], axis=3).astype(ml_dtypes.bfloat16)
    return idx_dram, w_dram


# ---------------- device kernel ----------------
def _build_bass(nit_lim=None):
    """nit_lim > 384 wraps (for slope timing); real workload is NIT=384."""
    nc = bacc.Bacc("TRN2", num_swdge_queues=4)
    tab = nc.dram_tensor("tab", [3, J, NROWS, 128], BF16, kind="ExternalInput")
    idx = nc.dram_tensor("idx", [J, NSB, 128, NIDX // 16], I16, kind="ExternalInput")
    w8 = nc.dram_tensor("w8", [J, NSB, 128, WLEN], BF16, kind="ExternalInput")
    # device-native layout [np, sb, nb, j]; host transposes to [NPTS, J]
    out = nc.dram_tensor("out", [128, NSB, NB, J], F32, kind="ExternalOutput")

    NIT = J * NSB  # 384 iterations, j outer / sb inner
    if nit_lim is not None:
        NIT = nit_lim
    D = DEPTH
    NCALL = len(GCALLS)

    from contextlib import ExitStack
    with ExitStack() as ctx:
        dst = ctx.enter_context(nc.sbuf_tensor("dst", [128, D, NSLOT, 128], BF16))
        idxs = ctx.enter_context(nc.sbuf_tensor("idxs", [128, D, NIDX // 16], I16))
        w8t = ctx.enter_context(nc.sbuf_tensor("w8t", [128, D, WLEN], BF16))
        wprod = ctx.enter_context(nc.sbuf_tensor("wprod", [128, 3072], BF16))
        t1 = ctx.enter_context(nc.sbuf_tensor("t1", [128, 1536], BF16))
        pf = ctx.enter_context(nc.sbuf_tensor("pf", [128, 768], F32))
        lft = ctx.enter_context(nc.sbuf_tensor("lft", [128, 1536], BF16))
        lf = ctx.enter_context(nc.sbuf_tensor("lf", [128, 768], F32))
        prod = ctx.enter_context(nc.sbuf_tensor("prod", [128, NB, 3, 16], F32))
        outt = ctx.enter_context(nc.sbuf_tensor("outt", [128, NSB, NB, J], F32))
        s_gat = [ctx.enter_context(nc.semaphore(f"s_gat{i}")) for i in range(D)]
        s_idx = [ctx.enter_context(nc.semaphore(f"s_idx{i}")) for i in range(D)]
        s_w8 = [ctx.enter_context(nc.semaphore(f"s_w8{i}")) for i in range(D)]
        s_cmb = ctx.enter_context(nc.semaphore("s_cmb"))
        s_relu = ctx.enter_context(nc.semaphore("s_relu"))
        s_out = ctx.enter_context(nc.semaphore("s_out"))
        s_v = ctx.enter_context(nc.semaphore("s_v"))
        block = ctx.enter_context(nc.Block())

        @block.gpsimd
        def _(gpsimd):
            gpsimd.load_library(mlp)
            for it in range(NIT):
                e = it % 384
                j = e // NSB
                b = it % D
                if it >= D:
                    # dst[b] free once compute(it-D) done
                    gpsimd.wait_ge(s_cmb, it - (D - 1))
                gpsimd.wait_ge(s_idx[b], 16 * (it // D + 1))
                for (p, lo, hi, q) in GCALLS:
                    s0 = p * NB + lo
                    s1 = p * NB + hi
                    n = (hi - lo) * 128
                    gpsimd.dma_gather(
                        dst[:, b, s0:s1, :], tab[p, j],
                        idxs[:, b, s0 * 8:s1 * 8],
                        n, n, 128, single_packet=False, queue_num=q,
                    ).then_inc(s_gat[b], 16)

        @block.sync
        def _(sync):
            for it in range(NIT):
                e = it % 384
                j, sb = e // NSB, e % NSB
                b = it % D
                if it >= D:
                    # w8t[b] free once compute(it-D) done
                    sync.wait_ge(s_cmb, it - (D - 1))
                    # idxs[b] consumed once gather(it-D) completed
                    sync.wait_ge(s_gat[b], 16 * NCALL * (it // D))
                sync.dma_start(idxs[:, b, :], idx[j, sb]).then_inc(s_idx[b], 16)
                sync.dma_start(w8t[:, b, :], w8[j, sb]).then_inc(s_w8[b], 16)
            # final output DMA after relu (contiguous, same layout)
            sync.wait_ge(s_relu, 1)
            sync.dma_start(out[:], outt[:]).then_inc(s_out, 16)
            sync.wait_ge(s_out, 16)

        @block.vector
        def _(vector):
            sv = 0

            def emit(inst):
                nonlocal sv
                sv += 1
                inst.then_inc(s_v, 1)

            def barrier():
                vector.wait_ge(s_v, sv)

            emit(vector.memset(outt[:].rearrange("P a b c -> P (a b c)"), 0.0))
            barrier()
            for it in range(NIT):
                e = it % 384
                j, sb = e // NSB, e % NSB
                b = it % D
                vector.wait_ge(s_gat[b], 16 * NCALL * (it // D + 1))
                vector.wait_ge(s_w8[b], 16 * (it // D + 1))
                # plane corners [3p,16nb | 16c,4yx] * wp [3p,16nb,4] bcast c
                in0 = dst[:, b, :, 0:64].rearrange(
                    "P (p nb) (c r) -> P p nb c r", p=3, c=16)
                in1 = w8t[:, b, 0:192].rearrange(
                    "P (p nb r) -> P p nb r", p=3, nb=NB
                ).unsqueeze(3).broadcast_to([128, 3, NB, 16, 4])
                wv = wprod[:].rearrange(
                    "P (p nb c r) -> P p nb c r", p=3, nb=NB, c=16)
                emit(vector.tensor_tensor(wv, in0, in1, mybir.AluOpType.mult))
                # line taps [3row,16nb | 16c,2] * wl [3row,16nb,2] bcast c
                li0 = dst[:, b, :, 64:96].rearrange(
                    "P (p nb) (c r) -> P p nb c r", p=3, c=16)
                li1 = w8t[:, b, 192:288].rearrange(
                    "P (p nb r) -> P p nb r", p=3, nb=NB
                ).unsqueeze(3).broadcast_to([128, 3, NB, 16, 2])
                lv = lft[:].rearrange(
                    "P (p nb c r) -> P p nb c r", p=3, nb=NB, c=16)
                emit(vector.tensor_tensor(lv, li0, li1, mybir.AluOpType.mult))
                barrier()
                # plane tree 4 -> 2 -> 1 ; line tree 2 -> 1
                w3 = wprod[:].rearrange("P (m r) -> P m r", r=4)
                t1v = t1[:].rearrange("P (m r) -> P m r", r=2)
                emit(vector.tensor_tensor(t1v, w3[:, :, 0:2], w3[:, :, 2:4],
                                          mybir.AluOpType.add))
                l3 = lft[:].rearrange("P (m r) -> P m r", r=2)
                emit(vector.tensor_tensor(lf[:], l3[:, :, 0], l3[:, :, 1],
                                          mybir.AluOpType.add))
                barrier()
                emit(vector.tensor_tensor(pf[:], t1v[:, :, 0], t1v[:, :, 1],
                                          mybir.AluOpType.add))
                barrier()
                # prod[nb, p, c] = pf[p, nb, c] * lf[rowof(p), nb, c]
                pfv = pf[:].rearrange("P (p nb c) -> P p nb c", p=3, c=16)
                lfv = lf[:].rearrange("P (p nb c) -> P p nb c", p=3, c=16)
                prodv = prod[:].rearrange("P nb p c -> P p nb c")
                emit(vector.tensor_tensor(
                    prodv[:, 0:2], pfv[:, 0:2], lfv[:, 1:3],
                    mybir.AluOpType.mult))
                emit(vector.tensor_tensor(
                    prodv[:, 2], pfv[:, 2], lfv[:, 0], mybir.AluOpType.mult))
                barrier()
                vector.tensor_reduce(
                    outt[:, sb, :, j],
                    prod[:].rearrange("P nb p c -> P nb (p c)"),
                    mybir.AxisListType.X, mybir.AluOpType.add,
                ).then_inc(s_cmb, 1)
            vector.wait_ge(s_cmb, NIT)
            of = outt[:].rearrange("P a b c -> P (a b c)")
            vector.tensor_scalar_max(of, of, 0.0).then_inc(s_relu, 1)

    nc.compile()
    return nc


# ---------------- entry point ----------------
def prepare_in_maps(inputs):
    planes = [np.asarray(inputs[f"plane{i}"]) for i in range(3)]
    lines = [np.asarray(inputs[f"line{i}"]) for i in range(3)]
    tab = _build_tables(planes, lines)
    idx3, wp, wl = _coords_weights(
        np.asarray(inputs["xyz"]), np.asarray(inputs["transforms"]))
    in_maps = []
    for k in range(N_CORES):
        s = slice(k * NPTS, (k + 1) * NPTS)
        idx_d, w_d = _pack_core(idx3[s], wp[s], wl[s])
        in_maps.append({"tab": tab, "idx": idx_d, "w8": w_d})
    return in_maps


def kernel(xyz, transforms, plane0, plane1, plane2, line0, line1, line2):
    in_maps = prepare_in_maps(dict(
        xyz=xyz, transforms=transforms, plane0=plane0, plane1=plane1,
        plane2=plane2, line0=line0, line1=line1, line2=line2))

    if "nc" not in _CACHE:
        _CACHE["nc"] = _build_bass()
    nc = _CACHE["nc"]

    _CACHE["in_maps"] = in_maps
    res = run_bass_kernel_spmd(nc, in_maps, core_ids=list(range(N_CORES)))
    outs = []
    for r in res.results:
        o = np.asarray(r["out"]).reshape(128, NSB, NB, J)
        outs.append(o.transpose(1, 2, 0, 3).reshape(NPTS, J))
    return np.concatenate(outs, axis=0).astype(np.float32)


if __name__ == "__main__":
    rng = np.random.default_rng(0)
    xyz = (rng.random((N_TOTAL, 3), np.float32) * 2 - 1).astype(np.float32)
    tr = (np.eye(4, dtype=np.float32)[None]
          + 0.05 * rng.standard_normal((J, 4, 4)).astype(np.float32))
    pl = [(0.032 * rng.standard_normal((J, C, G, G))).astype(np.float32) for _ in range(3)]
    ln = [(0.032 * rng.standard_normal((J, C, G))).astype(np.float32) for _ in range(3)]
    o = kernel(xyz, tr, pl[0], pl[1], pl[2], ln[0], ln[1], ln[2])
    print(o.shape, o.dtype, float(o.max()))


# revision 12
# speedup vs baseline: 1.1787x; 1.0018x over previous
"""Trainium2 Bass kernel for nn_BWCaster_86337432584570 (embedding_lookup), v3.

sigma[n,j] = relu( sum_p sum_c bilinear(plane_p[j])[c] * linear(line_p[j])[c] )

v3 design: 3 fused 256B gather rows per (point, joint) — each plane's row
carries a line's 2 exact taps in its padding (the line coordinate equals an
exact row/col index of some plane):
  row0 (y=|g1|, x=|g0|): P0[c,2y,2x] (64) | L2[c, x:x+2] (32) | pad
  row1 (y=|g2|, x=|g0|): P1[c,2y,2x]      | L0[c, y:y+2]      | pad
  row2 (y=|g2|, x=|g1|): P2[c,2y,2x]      | L1[c, x:x+2]      | pad
Gathers run on 4 SWDGE queues (6 calls/iter, 12 slots per queue) — the
per-queue descriptor ring is the bottleneck (~7-9 ns/desc/queue).
"""
import sys
import numpy as np
import ml_dtypes

sys.path.insert(0, "/opt/trn_rl_repo")

import concourse.bass as bass
import concourse.bacc as bacc
import concourse.mybir as mybir
from concourse.bass_utils import run_bass_kernel_spmd
from concourse.library_config import mlp

# ---------------- problem constants (hardcoded) ----------------
N_TOTAL, J, C, G = 262144, 24, 16, 128
N_CORES = 8
NPTS = N_TOTAL // N_CORES          # 32768 points per core
SB = 2048                          # points per superblock
NSB = NPTS // SB                   # 16
NB = SB // 128                     # 16 sub-blocks of 128 points
NSLOT = 3 * NB                     # 48 gather slots per (j, superblock)
NIDX = NSLOT * 128                 # 6144 indices per (j, superblock)
DEPTH = 4                          # pipeline depth (gather buffers)
NROWS = 127 * 127                  # 16129 rows per (j, p) table
WLEN = 3 * NB * 4 + 3 * NB * 2     # 288 weights per partition-point
# gather sub-calls: (p, slot_lo, slot_hi, queue)
GCALLS = [(0, 0, 12, 0), (0, 12, 16, 1),
          (1, 0, 8, 1), (1, 8, 16, 2),
          (2, 0, 4, 2), (2, 4, 16, 3)]
BF16 = mybir.dt.bfloat16
F32 = mybir.dt.float32
I16 = mybir.dt.int16

_CACHE = {}


# ---------------- host-side prep ----------------
def _build_tables(planes, lines):
    """tab [3, J, NROWS, 128] bf16; row (y,x) of table p:
    [16c x (2y,2x)] plane corners | [16c x 2] line taps | 32 pad."""
    tab = np.zeros((3, J, NROWS, 128), dtype=np.float32)
    for p in range(3):
        sw = np.lib.stride_tricks.sliding_window_view(planes[p], (2, 2), axis=(2, 3))
        # [J, C, 127, 127, 2, 2] -> [J, y, x, C, 2, 2]
        tab[p, :, :, 0:64] = sw.transpose(0, 2, 3, 1, 4, 5).reshape(J, NROWS, 64)
    # line tails (2 exact taps, broadcast over the unused row coordinate)
    l2 = np.lib.stride_tricks.sliding_window_view(lines[2], 2, axis=2)  # [J,C,127,2]
    t0 = np.broadcast_to(l2.transpose(0, 2, 1, 3)[:, None, :, :, :],
                         (J, 127, 127, C, 2))                  # bcast over y
    tab[0, :, :, 64:96] = t0.reshape(J, NROWS, 32)
    l0 = np.lib.stride_tricks.sliding_window_view(lines[0], 2, axis=2)
    t1 = np.broadcast_to(l0.transpose(0, 2, 1, 3)[:, :, None, :, :],
                         (J, 127, 127, C, 2))                  # bcast over x
    tab[1, :, :, 64:96] = t1.reshape(J, NROWS, 32)
    l1 = np.lib.stride_tricks.sliding_window_view(lines[1], 2, axis=2)
    t2 = np.broadcast_to(l1.transpose(0, 2, 1, 3)[:, None, :, :, :],
                         (J, 127, 127, C, 2))                  # bcast over y
    tab[2, :, :, 64:96] = t2.reshape(J, NROWS, 32)
    return tab.astype(ml_dtypes.bfloat16)


def _coords_weights(xyz, transforms):
    """Returns idx3 [N,J,3] int16, wp [N,J,3,4] f32, wl [N,J,3,2] f32.
    wl is in ROW order (row0=L2, row1=L0, row2=L1)."""
    N = xyz.shape[0]
    xyzh = np.concatenate([xyz, np.ones((N, 1), np.float32)], axis=1)
    pts = np.einsum('jab,nb->nja', transforms[:, :3, :].astype(np.float32), xyzh)
    coord = (pts * np.float32(2.0 / 3.0) + np.float32(1.0)) * np.float32(0.5 * (G - 1))
    c0 = np.floor(coord).astype(np.int32)          # [N,J,3] per-axis cell
    fr = (coord - c0).astype(np.float32)
    c0c = np.clip(c0, 0, 126)                      # safety; margin makes this a no-op

    # (y_axis, x_axis) per plane row: row0=(1,0), row1=(2,0), row2=(2,1)
    YX = [(1, 0), (2, 0), (2, 1)]
    idx3 = np.empty((N, J, 3), np.int16)
    wp = np.empty((N, J, 3, 4), np.float32)
    wl = np.empty((N, J, 3, 2), np.float32)
    for p, (ya, xa) in enumerate(YX):
        y0, x0 = c0c[:, :, ya], c0c[:, :, xa]
        fy, fx = fr[:, :, ya], fr[:, :, xa]
        idx3[:, :, p] = (y0 * 127 + x0).astype(np.int16)
        wy0, wx0 = 1.0 - fy, 1.0 - fx
        wp[:, :, p, 0] = wy0 * wx0
        wp[:, :, p, 1] = wy0 * fx
        wp[:, :, p, 2] = fy * wx0
        wp[:, :, p, 3] = fy * fx
    # line taps: row0 tail = L2 @ g0 (axis 0), row1 = L0 @ g2, row2 = L1 @ g1
    for p, ax in enumerate([0, 2, 1]):
        f = fr[:, :, ax]
        wl[:, :, p, 0] = 1.0 - f
        wl[:, :, p, 1] = f
    return idx3, wp, wl


def _pack_core(idx3, wp, wl):
    """idx3 [NPTS,J,3] -> idx dram [J,NSB,128,NIDX//16] int16 (wrapped+replicated)
    wp/wl -> w dram [J,NSB,128,WLEN] bf16: [3p,16nb,4 | 3row,16nb,2]."""
    u = idx3.reshape(NSB, NB, 128, J, 3)
    arr = u.transpose(3, 0, 4, 1, 2).reshape(J, NSB, NSLOT * 128)
    wrapped = arr.reshape(J, NSB, NIDX // 16, 16).transpose(0, 1, 3, 2)
    idx_dram = np.broadcast_to(wrapped[:, :, None, :, :], (J, NSB, 8, 16, NIDX // 16))
    idx_dram = np.ascontiguousarray(idx_dram).reshape(J, NSB, 128, NIDX // 16)

    a = wp.reshape(NSB, NB, 128, J, 3, 4).transpose(3, 0, 2, 4, 1, 5)  # [j,sb,np,p,nb,4]
    b = wl.reshape(NSB, NB, 128, J, 3, 2).transpose(3, 0, 2, 4, 1, 5)  # [j,sb,np,r,nb,2]
    w_dram = np.concatenate([
        np.ascontiguousarray(a).reshape(J, NSB, 128, 192),
        np.ascontiguousarray(b).reshape(J, NSB, 128, 96),
    ], axis=3).astype(ml_dtypes.bfloat16)
    return idx_dram, w_dram


# ---------------- device kernel ----------------
def _build_bass(nit_lim=None):
    """nit_lim > 384 wraps (for slope timing); real workload is NIT=384."""
    nc = bacc.Bacc("TRN2", num_swdge_queues=4)
    tab = nc.dram_tensor("tab", [3, J, NROWS, 128], BF16, kind="ExternalInput")
    idx = nc.dram_tensor("idx", [J, NSB, 128, NIDX // 16], I16, kind="ExternalInput")
    w8 = nc.dram_tensor("w8", [J, NSB, 128, WLEN], BF16, kind="ExternalInput")
    # device-native layout [np, sb, nb, j]; host transposes to [NPTS, J]
    out = nc.dram_tensor("out", [128, NSB, NB, J], F32, kind="ExternalOutput")

    NIT = J * NSB  # 384 iterations, j outer / sb inner
    if nit_lim is not None:
        NIT = nit_lim
    D = DEPTH
    NCALL = len(GCALLS)

    from contextlib import ExitStack
    with ExitStack() as ctx:
        dst = ctx.enter_context(nc.sbuf_tensor("dst", [128, D, NSLOT, 128], BF16))
        idxs = ctx.enter_context(nc.sbuf_tensor("idxs", [128, D, NIDX // 16], I16))
        w8t = ctx.enter_context(nc.sbuf_tensor("w8t", [128, D, WLEN], BF16))
        wprod = ctx.enter_context(nc.sbuf_tensor("wprod", [128, 3072], BF16))
        t1 = ctx.enter_context(nc.sbuf_tensor("t1", [128, 1536], BF16))
        pf = ctx.enter_context(nc.sbuf_tensor("pf", [128, 768], F32))
        lft = ctx.enter_context(nc.sbuf_tensor("lft", [128, 1536], BF16))
        lf = ctx.enter_context(nc.sbuf_tensor("lf", [128, 768], F32))
        prod = ctx.enter_context(nc.sbuf_tensor("prod", [128, NB, 3, 16], F32))
        outt = ctx.enter_context(nc.sbuf_tensor("outt", [128, NSB, NB, J], F32))
        s_gat = [ctx.enter_context(nc.semaphore(f"s_gat{i}")) for i in range(D)]
        s_idx = [ctx.enter_context(nc.semaphore(f"s_idx{i}")) for i in range(D)]
        s_w8 = [ctx.enter_context(nc.semaphore(f"s_w8{i}")) for i in range(D)]
        s_cmb = ctx.enter_context(nc.semaphore("s_cmb"))
        s_relu = ctx.enter_context(nc.semaphore("s_relu"))
        s_out = ctx.enter_context(nc.semaphore("s_out"))
        s_v = ctx.enter_context(nc.semaphore("s_v"))
        block = ctx.enter_context(nc.Block())

        @block.gpsimd
        def _(gpsimd):
            gpsimd.load_library(mlp)
            for it in range(NIT):
                e = it % 384
                j = e // NSB
                b = it % D
                if it >= D:
                    # dst[b] free once compute(it-D) done
                    gpsimd.wait_ge(s_cmb, it - (D - 1))
                gpsimd.wait_ge(s_idx[b], 16 * (it // D + 1))
                for (p, lo, hi, q) in GCALLS:
                    s0 = p * NB + lo
                    s1 = p * NB + hi
                    n = (hi - lo) * 128
                    gpsimd.dma_gather(
                        dst[:, b, s0:s1, :], tab[p, j],
                        idxs[:, b, s0 * 8:s1 * 8],
                        n, n, 128, single_packet=False, queue_num=q,
                    ).then_inc(s_gat[b], 16)

        @block.sync
        def _(sync):
            for it in range(NIT):
                e = it % 384
                j, sb = e // NSB, e % NSB
                b = it % D
                if it >= D:
                    # w8t[b] free once compute(it-D) done
                    sync.wait_ge(s_cmb, it - (D - 1))
                    # idxs[b] consumed once gather(it-D) completed
                    sync.wait_ge(s_gat[b], 16 * NCALL * (it // D))
                sync.dma_start(idxs[:, b, :], idx[j, sb]).then_inc(s_idx[b], 16)
                sync.dma_start(w8t[:, b, :], w8[j, sb]).then_inc(s_w8[b], 16)
            # final output DMA after relu (contiguous, same layout)
            sync.wait_ge(s_relu, 1)
            sync.dma_start(out[:], outt[:]).then_inc(s_out, 16)
            sync.wait_ge(s_out, 16)

        @block.vector
        def _(vector):
            sv = 0

            def emit(inst):
                nonlocal sv
                sv += 1
                inst.then_inc(s_v, 1)

            def barrier():
                vector.wait_ge(s_v, sv)

            emit(vector.memset(outt[:].rearrange("P a b c -> P (a b c)"), 0.0))
            barrier()
            for it in range(NIT):
                e = it % 384
                j, sb = e // NSB, e % NSB
                b = it % D
                vector.wait_ge(s_gat[b], 16 * NCALL * (it // D + 1))
                vector.wait_ge(s_w8[b], 16 * (it // D + 1))
                # plane corners [3p,16nb | 16c,4yx] * wp [3p,16nb,4] bcast c
                in0 = dst[:, b, :, 0:64].rearrange(
                    "P (p nb) (c r) -> P p nb c r", p=3, c=16)
                in1 = w8t[:, b, 0:192].rearrange(
                    "P (p nb r) -> P p nb r", p=3, nb=NB
                ).unsqueeze(3).broadcast_to([128, 3, NB, 16, 4])
                wv = wprod[:].rearrange(
                    "P (p nb c r) -> P p nb c r", p=3, nb=NB, c=16)
                emit(vector.tensor_tensor(wv, in0, in1, mybir.AluOpType.mult))
                # line taps [3row,16nb | 16c,2] * wl [3row,16nb,2] bcast c
                li0 = dst[:, b, :, 64:96].rearrange(
                    "P (p nb) (c r) -> P p nb c r", p=3, c=16)
                li1 = w8t[:, b, 192:288].rearrange(
                    "P (p nb r) -> P p nb r", p=3, nb=NB
                ).unsqueeze(3).broadcast_to([128, 3, NB, 16, 2])
                lv = lft[:].rearrange(
                    "P (p nb c r) -> P p nb c r", p=3, nb=NB, c=16)
                emit(vector.tensor_tensor(lv, li0, li1, mybir.AluOpType.mult))
                barrier()
                # plane tree 4 -> 2 -> 1 ; line tree 2 -> 1
                w3 = wprod[:].rearrange("P (m r) -> P m r", r=4)
                t1v = t1[:].rearrange("P (m r) -> P m r", r=2)
                emit(vector.tensor_tensor(t1v, w3[:, :, 0:2], w3[:, :, 2:4],
                                          mybir.AluOpType.add))
                l3 = lft[:].rearrange("P (m r) -> P m r", r=2)
                emit(vector.tensor_tensor(lf[:], l3[:, :, 0], l3[:, :, 1],
                                          mybir.AluOpType.add))
                barrier()
                emit(vector.tensor_tensor(pf[:], t1v[:, :, 0], t1v[:, :, 1],
                                          mybir.AluOpType.add))
                barrier()
                # prod[nb, p, c] = pf[p, nb, c] * lf[rowof(p), nb, c]
                pfv = pf[:].rearrange("P (p nb c) -> P p nb c", p=3, c=16)
                lfv = lf[:].rearrange("P (p nb c) -> P p nb c", p=3, c=16)
                for p, row in enumerate([1, 2, 0]):
                    emit(vector.tensor_tensor(
                        prod[:, :, p, :], pfv[:, p], lfv[:, row],
                        mybir.AluOpType.mult))
                barrier()
                vector.tensor_reduce(
                    outt[:, sb, :, j],
                    prod[:].rearrange("P nb p c -> P nb (p c)"),
                    mybir.AxisListType.X, mybir.AluOpType.add,
                ).then_inc(s_cmb, 1)
            vector.wait_ge(s_cmb, NIT)
            of = outt[:].rearrange("P a b c -> P (a b c)")
            vector.tensor_scalar_max(of, of, 0.0).then_inc(s_relu, 1)

    nc.compile()
    return nc


# ---------------- entry point ----------------
def prepare_in_maps(inputs):
    planes = [np.asarray(inputs[f"plane{i}"]) for i in range(3)]
    lines = [np.asarray(inputs[f"line{i}"]) for i in range(3)]
    tab = _build_tables(planes, lines)
    idx3, wp, wl = _coords_weights(
        np.asarray(inputs["xyz"]), np.asarray(inputs["transforms"]))
    in_maps = []
    for k in range(N_CORES):
        s = slice(k * NPTS, (k + 1) * NPTS)
        idx_d, w_d = _pack_core(idx3[s], wp[s], wl[s])
        in_maps.append({"tab": tab, "idx": idx_d, "w8": w_d})
    return in_maps


def kernel(xyz, transforms, plane0, plane1, plane2, line0, line1, line2):
    in_maps = prepare_in_maps(dict(
        xyz=xyz, transforms=transforms, plane0=plane0, plane1=plane1,
        plane2=plane2, line0=line0, line1=line1, line2=line2))

    if "nc" not in _CACHE:
        _CACHE["nc"] = _build_bass()
    nc = _CACHE["nc"]

    _CACHE["in_maps"] = in_maps
    res = run_bass_kernel_spmd(nc, in_maps, core_ids=list(range(N_CORES)))
    outs = []
    for r in res.results:
        o = np.asarray(r["out"]).reshape(128, NSB, NB, J)
        outs.append(o.transpose(1, 2, 0, 3).reshape(NPTS, J))
    return np.concatenate(outs, axis=0).astype(np.float32)


if __name__ == "__main__":
    rng = np.random.default_rng(0)
    xyz = (rng.random((N_TOTAL, 3), np.float32) * 2 - 1).astype(np.float32)
    tr = (np.eye(4, dtype=np.float32)[None]
          + 0.05 * rng.standard_normal((J, 4, 4)).astype(np.float32))
    pl = [(0.032 * rng.standard_normal((J, C, G, G))).astype(np.float32) for _ in range(3)]
    ln = [(0.032 * rng.standard_normal((J, C, G))).astype(np.float32) for _ in range(3)]
    o = kernel(xyz, tr, pl[0], pl[1], pl[2], ln[0], ln[1], ln[2])
    print(o.shape, o.dtype, float(o.max()))


# revision 18
# speedup vs baseline: 1.2411x; 1.0529x over previous
"""Trainium2 Bass kernel for nn_BWCaster_86337432584570 (embedding_lookup), v3.

sigma[n,j] = relu( sum_p sum_c bilinear(plane_p[j])[c] * linear(line_p[j])[c] )

v3 design: 3 fused 256B gather rows per (point, joint) — each plane's row
carries a line's 2 exact taps in its padding (the line coordinate equals an
exact row/col index of some plane):
  row0 (y=|g1|, x=|g0|): P0[c,2y,2x] (64) | L2[c, x:x+2] (32) | pad
  row1 (y=|g2|, x=|g0|): P1[c,2y,2x]      | L0[c, y:y+2]      | pad
  row2 (y=|g2|, x=|g1|): P2[c,2y,2x]      | L1[c, x:x+2]      | pad
Gathers run on 4 SWDGE queues (6 calls/iter, 12 slots per queue) — the
per-queue descriptor ring is the bottleneck (~7-9 ns/desc/queue).
"""
import sys
import numpy as np
import ml_dtypes

sys.path.insert(0, "/opt/trn_rl_repo")

import concourse.bass as bass
import concourse.bacc as bacc
import concourse.mybir as mybir
from concourse.bass_utils import run_bass_kernel_spmd
from concourse.library_config import mlp

# ---------------- problem constants (hardcoded) ----------------
N_TOTAL, J, C, G = 262144, 24, 16, 128
N_CORES = 8
NPTS = N_TOTAL // N_CORES          # 32768 points per core
SB = 2048                          # points per superblock
NSB = NPTS // SB                   # 16
NB = SB // 128                     # 16 sub-blocks of 128 points
NSLOT = 3 * NB                     # 48 gather slots per (j, superblock)
NIDX = NSLOT * 128                 # 6144 indices per (j, superblock)
DEPTH = 4                          # pipeline depth (gather buffers)
NROWS = 127 * 127                  # 16129 rows per (j, p) table
WLEN = 3 * NB * 4 + 3 * NB * 2     # 288 weights per partition-point
# Slot order is (p0, p2, p1); tables for p0 and p2 are merged into one
# 32258-row index space (still int16) so a gather call can span both.
# gather sub-calls: (table, slot_lo, slot_hi, queue); table 0 = merged
# p0|p2 (rows 0..2*NROWS), table 1 = p1 (rows 0..NROWS).
GCALLS = [(0, 0, 12, 0), (0, 12, 24, 1), (0, 24, 32, 2),
          (1, 32, 36, 2), (1, 36, 48, 3)]
BF16 = mybir.dt.bfloat16
F32 = mybir.dt.float32
I16 = mybir.dt.int16

_CACHE = {}


# ---------------- host-side prep ----------------
def _build_tables(planes, lines):
    """tab [J, 3*NROWS, 128] bf16 in slot order (p0, p2, p1); row (y,x) of
    plane p: [16c x (2y,2x)] corners | [16c x 2] line taps | 32 pad."""
    tab = np.zeros((3, J, NROWS, 128), dtype=np.float32)
    for p in range(3):
        sw = np.lib.stride_tricks.sliding_window_view(planes[p], (2, 2), axis=(2, 3))
        # [J, C, 127, 127, 2, 2] -> [J, y, x, C, 2, 2]
        tab[p, :, :, 0:64] = sw.transpose(0, 2, 3, 1, 4, 5).reshape(J, NROWS, 64)
    # line tails (2 exact taps, broadcast over the unused row coordinate)
    l2 = np.lib.stride_tricks.sliding_window_view(lines[2], 2, axis=2)  # [J,C,127,2]
    t0 = np.broadcast_to(l2.transpose(0, 2, 1, 3)[:, None, :, :, :],
                         (J, 127, 127, C, 2))                  # bcast over y
    tab[0, :, :, 64:96] = t0.reshape(J, NROWS, 32)
    l0 = np.lib.stride_tricks.sliding_window_view(lines[0], 2, axis=2)
    t1 = np.broadcast_to(l0.transpose(0, 2, 1, 3)[:, :, None, :, :],
                         (J, 127, 127, C, 2))                  # bcast over x
    tab[1, :, :, 64:96] = t1.reshape(J, NROWS, 32)
    l1 = np.lib.stride_tricks.sliding_window_view(lines[1], 2, axis=2)
    t2 = np.broadcast_to(l1.transpose(0, 2, 1, 3)[:, None, :, :, :],
                         (J, 127, 127, C, 2))                  # bcast over y
    tab[2, :, :, 64:96] = t2.reshape(J, NROWS, 32)
    # merge to [J, (p0|p2|p1) * NROWS, 128]
    tabm = np.stack([tab[0], tab[2], tab[1]], axis=1).reshape(J, 3 * NROWS, 128)
    return tabm.astype(ml_dtypes.bfloat16)


def _coords_weights(xyz, transforms):
    """Returns idx3 [N,J,3] int16, wp [N,J,3,4] f32, wl [N,J,3,2] f32.
    wl is in ROW order (row0=L2, row1=L0, row2=L1)."""
    N = xyz.shape[0]
    xyzh = np.concatenate([xyz, np.ones((N, 1), np.float32)], axis=1)
    pts = np.einsum('jab,nb->nja', transforms[:, :3, :].astype(np.float32), xyzh)
    coord = (pts * np.float32(2.0 / 3.0) + np.float32(1.0)) * np.float32(0.5 * (G - 1))
    c0 = np.floor(coord).astype(np.int32)          # [N,J,3] per-axis cell
    fr = (coord - c0).astype(np.float32)
    c0c = np.clip(c0, 0, 126)                      # safety; margin makes this a no-op

    # (y_axis, x_axis) per plane row: row0=(1,0), row1=(2,0), row2=(2,1)
    YX = [(1, 0), (2, 0), (2, 1)]
    idx3 = np.empty((N, J, 3), np.int16)
    wp = np.empty((N, J, 3, 4), np.float32)
    wl = np.empty((N, J, 3, 2), np.float32)
    for p, (ya, xa) in enumerate(YX):
        y0, x0 = c0c[:, :, ya], c0c[:, :, xa]
        fy, fx = fr[:, :, ya], fr[:, :, xa]
        idx3[:, :, p] = (y0 * 127 + x0).astype(np.int16)
        wy0, wx0 = 1.0 - fy, 1.0 - fx
        wp[:, :, p, 0] = wy0 * wx0
        wp[:, :, p, 1] = wy0 * fx
        wp[:, :, p, 2] = fy * wx0
        wp[:, :, p, 3] = fy * fx
    # line taps: row0 tail = L2 @ g0 (axis 0), row1 = L0 @ g2, row2 = L1 @ g1
    for p, ax in enumerate([0, 2, 1]):
        f = fr[:, :, ax]
        wl[:, :, p, 0] = 1.0 - f
        wl[:, :, p, 1] = f
    return idx3, wp, wl


def _pack_core(idx3, wp, wl):
    """Slot order (p0, p2, p1); p2 indices offset by NROWS (merged table).
    idx3 [NPTS,J,3] -> idx dram [J,NSB,128,NIDX//16] int16 (wrapped+replicated)
    wp/wl -> w dram [J,NSB,128,WLEN] bf16: [3slot,16nb,4 | 3slot,16nb,2]."""
    SLOT_P = [0, 2, 1]
    idx3s = idx3[:, :, SLOT_P].astype(np.int32)
    idx3s[:, :, 1] += NROWS                       # p2 rows live at offset NROWS
    idx3s = idx3s.astype(np.int16)
    u = idx3s.reshape(NSB, NB, 128, J, 3)
    arr = u.transpose(3, 0, 4, 1, 2).reshape(J, NSB, NSLOT * 128)
    wrapped = arr.reshape(J, NSB, NIDX // 16, 16).transpose(0, 1, 3, 2)
    idx_dram = np.broadcast_to(wrapped[:, :, None, :, :], (J, NSB, 8, 16, NIDX // 16))
    idx_dram = np.ascontiguousarray(idx_dram).reshape(J, NSB, 128, NIDX // 16)

    # weights follow slot order; line-tap weights follow the slot's row tail:
    # slot0 = row-p0 (L2), slot1 = row-p2 (L1), slot2 = row-p1 (L0)
    wps = wp[:, :, SLOT_P, :]
    wls = wl[:, :, [0, 2, 1], :]
    a = wps.reshape(NSB, NB, 128, J, 3, 4).transpose(3, 0, 2, 4, 1, 5)
    b = wls.reshape(NSB, NB, 128, J, 3, 2).transpose(3, 0, 2, 4, 1, 5)
    w_dram = np.concatenate([
        np.ascontiguousarray(a).reshape(J, NSB, 128, 192),
        np.ascontiguousarray(b).reshape(J, NSB, 128, 96),
    ], axis=3).astype(ml_dtypes.bfloat16)
    return idx_dram, w_dram


# ---------------- device kernel ----------------
def _build_bass(nit_lim=None):
    """nit_lim > 384 wraps (for slope timing); real workload is NIT=384."""
    nc = bacc.Bacc("TRN2", num_swdge_queues=4)
    tab = nc.dram_tensor("tab", [J, 3 * NROWS, 128], BF16, kind="ExternalInput")
    idx = nc.dram_tensor("idx", [J, NSB, 128, NIDX // 16], I16, kind="ExternalInput")
    w8 = nc.dram_tensor("w8", [J, NSB, 128, WLEN], BF16, kind="ExternalInput")
    # device-native layout [np, sb, nb, j]; host transposes to [NPTS, J]
    out = nc.dram_tensor("out", [128, NSB, NB, J], F32, kind="ExternalOutput")

    NIT = J * NSB  # 384 iterations, j outer / sb inner
    if nit_lim is not None:
        NIT = nit_lim
    D = DEPTH
    NCALL = len(GCALLS)

    from contextlib import ExitStack
    with ExitStack() as ctx:
        dst = ctx.enter_context(nc.sbuf_tensor("dst", [128, D, NSLOT, 128], BF16))
        idxs = ctx.enter_context(nc.sbuf_tensor("idxs", [128, D, NIDX // 16], I16))
        w8t = ctx.enter_context(nc.sbuf_tensor("w8t", [128, D, WLEN], BF16))
        wprod = ctx.enter_context(nc.sbuf_tensor("wprod", [128, 3072], BF16))
        t1 = ctx.enter_context(nc.sbuf_tensor("t1", [128, 1536], BF16))
        pf = ctx.enter_context(nc.sbuf_tensor("pf", [128, 768], F32))
        lft = ctx.enter_context(nc.sbuf_tensor("lft", [128, 1536], BF16))
        lf = ctx.enter_context(nc.sbuf_tensor("lf", [128, 768], F32))
        prod = ctx.enter_context(nc.sbuf_tensor("prod", [128, NB, 3, 16], F32))
        outt = ctx.enter_context(nc.sbuf_tensor("outt", [128, NSB, NB, J], F32))
        s_gat = [ctx.enter_context(nc.semaphore(f"s_gat{i}")) for i in range(D)]
        s_idx = [ctx.enter_context(nc.semaphore(f"s_idx{i}")) for i in range(D)]
        s_w8 = [ctx.enter_context(nc.semaphore(f"s_w8{i}")) for i in range(D)]
        s_cmb = ctx.enter_context(nc.semaphore("s_cmb"))
        s_relu = ctx.enter_context(nc.semaphore("s_relu"))
        s_out = ctx.enter_context(nc.semaphore("s_out"))
        s_v = ctx.enter_context(nc.semaphore("s_v"))
        block = ctx.enter_context(nc.Block())

        @block.gpsimd
        def _(gpsimd):
            gpsimd.load_library(mlp)
            for it in range(NIT):
                e = it % 384
                j = e // NSB
                b = it % D
                if it >= D:
                    # dst[b] free once compute(it-D) done
                    gpsimd.wait_ge(s_cmb, it - (D - 1))
                gpsimd.wait_ge(s_idx[b], 16 * (it // D + 1))
                for (t, s0, s1, q) in GCALLS:
                    n = (s1 - s0) * 128
                    src = tab[j, 0:2 * NROWS] if t == 0 \
                        else tab[j, 2 * NROWS:3 * NROWS]
                    gpsimd.dma_gather(
                        dst[:, b, s0:s1, :], src,
                        idxs[:, b, s0 * 8:s1 * 8],
                        n, n, 128, single_packet=False, queue_num=q,
                    ).then_inc(s_gat[b], 16)

        @block.sync
        def _(sync):
            for it in range(NIT):
                e = it % 384
                j, sb = e // NSB, e % NSB
                b = it % D
                if it >= D:
                    # w8t[b] free once compute(it-D) done
                    sync.wait_ge(s_cmb, it - (D - 1))
                    # idxs[b] consumed once gather(it-D) completed
                    sync.wait_ge(s_gat[b], 16 * NCALL * (it // D))
                sync.dma_start(idxs[:, b, :], idx[j, sb]).then_inc(s_idx[b], 16)
                sync.dma_start(w8t[:, b, :], w8[j, sb]).then_inc(s_w8[b], 16)
            # final output DMA after relu (contiguous, same layout)
            sync.wait_ge(s_relu, 1)
            sync.dma_start(out[:], outt[:]).then_inc(s_out, 16)
            sync.wait_ge(s_out, 16)

        @block.vector
        def _(vector):
            sv = 0

            def emit(inst):
                nonlocal sv
                sv += 1
                inst.then_inc(s_v, 1)

            def barrier():
                vector.wait_ge(s_v, sv)

            emit(vector.memset(outt[:].rearrange("P a b c -> P (a b c)"), 0.0))
            barrier()
            for it in range(NIT):
                e = it % 384
                j, sb = e // NSB, e % NSB
                b = it % D
                vector.wait_ge(s_gat[b], 16 * NCALL * (it // D + 1))
                vector.wait_ge(s_w8[b], 16 * (it // D + 1))
                # plane corners [3p,16nb | 16c,4yx] * wp [3p,16nb,4] bcast c
                in0 = dst[:, b, :, 0:64].rearrange(
                    "P (p nb) (c r) -> P p nb c r", p=3, c=16)
                in1 = w8t[:, b, 0:192].rearrange(
                    "P (p nb r) -> P p nb r", p=3, nb=NB
                ).unsqueeze(3).broadcast_to([128, 3, NB, 16, 4])
                wv = wprod[:].rearrange(
                    "P (p nb c r) -> P p nb c r", p=3, nb=NB, c=16)
                emit(vector.tensor_tensor(wv, in0, in1, mybir.AluOpType.mult))
                # line taps [3row,16nb | 16c,2] * wl [3row,16nb,2] bcast c
                li0 = dst[:, b, :, 64:96].rearrange(
                    "P (p nb) (c r) -> P p nb c r", p=3, c=16)
                li1 = w8t[:, b, 192:288].rearrange(
                    "P (p nb r) -> P p nb r", p=3, nb=NB
                ).unsqueeze(3).broadcast_to([128, 3, NB, 16, 2])
                lv = lft[:].rearrange(
                    "P (p nb c r) -> P p nb c r", p=3, nb=NB, c=16)
                emit(vector.tensor_tensor(lv, li0, li1, mybir.AluOpType.mult))
                barrier()
                # plane tree 4 -> 2 -> 1 ; line tree 2 -> 1
                w3 = wprod[:].rearrange("P (m r) -> P m r", r=4)
                t1v = t1[:].rearrange("P (m r) -> P m r", r=2)
                emit(vector.tensor_tensor(t1v, w3[:, :, 0:2], w3[:, :, 2:4],
                                          mybir.AluOpType.add))
                l3 = lft[:].rearrange("P (m r) -> P m r", r=2)
                emit(vector.tensor_tensor(lf[:], l3[:, :, 0], l3[:, :, 1],
                                          mybir.AluOpType.add))
                barrier()
                emit(vector.tensor_tensor(pf[:], t1v[:, :, 0], t1v[:, :, 1],
                                          mybir.AluOpType.add))
                barrier()
                # pf slots (p0,p2,p1); lf slots (L2,L1,L0); pair pf_p with L_p
                pfv = pf[:].rearrange("P (p nb c) -> P p nb c", p=3, c=16)
                lfv = lf[:].rearrange("P (p nb c) -> P p nb c", p=3, c=16)
                prodv = prod[:].rearrange("P nb p c -> P p nb c")
                emit(vector.tensor_tensor(
                    prodv[:, 1:3], pfv[:, 1:3], lfv[:, 0:2],
                    mybir.AluOpType.mult))
                emit(vector.tensor_tensor(
                    prodv[:, 0], pfv[:, 0], lfv[:, 2], mybir.AluOpType.mult))
                barrier()
                vector.tensor_reduce(
                    outt[:, sb, :, j],
                    prod[:].rearrange("P nb p c -> P nb (p c)"),
                    mybir.AxisListType.X, mybir.AluOpType.add,
                ).then_inc(s_cmb, 1)
            vector.wait_ge(s_cmb, NIT)
            of = outt[:].rearrange("P a b c -> P (a b c)")
            vector.tensor_scalar_max(of, of, 0.0).then_inc(s_relu, 1)

    nc.compile()
    return nc


# ---------------- entry point ----------------
def prepare_in_maps(inputs):
    planes = [np.asarray(inputs[f"plane{i}"]) for i in range(3)]
    lines = [np.asarray(inputs[f"line{i}"]) for i in range(3)]
    tab = _build_tables(planes, lines)
    idx3, wp, wl = _coords_weights(
        np.asarray(inputs["xyz"]), np.asarray(inputs["transforms"]))
    in_maps = []
    for k in range(N_CORES):
        s = slice(k * NPTS, (k + 1) * NPTS)
        idx_d, w_d = _pack_core(idx3[s], wp[s], wl[s])
        in_maps.append({"tab": tab, "idx": idx_d, "w8": w_d})
    return in_maps


def kernel(xyz, transforms, plane0, plane1, plane2, line0, line1, line2):
    in_maps = prepare_in_maps(dict(
        xyz=xyz, transforms=transforms, plane0=plane0, plane1=plane1,
        plane2=plane2, line0=line0, line1=line1, line2=line2))

    if "nc" not in _CACHE:
        _CACHE["nc"] = _build_bass()
    nc = _CACHE["nc"]

    _CACHE["in_maps"] = in_maps
    res = run_bass_kernel_spmd(nc, in_maps, core_ids=list(range(N_CORES)))
    outs = []
    for r in res.results:
        o = np.asarray(r["out"]).reshape(128, NSB, NB, J)
        outs.append(o.transpose(1, 2, 0, 3).reshape(NPTS, J))
    return np.concatenate(outs, axis=0).astype(np.float32)


if __name__ == "__main__":
    rng = np.random.default_rng(0)
    xyz = (rng.random((N_TOTAL, 3), np.float32) * 2 - 1).astype(np.float32)
    tr = (np.eye(4, dtype=np.float32)[None]
          + 0.05 * rng.standard_normal((J, 4, 4)).astype(np.float32))
    pl = [(0.032 * rng.standard_normal((J, C, G, G))).astype(np.float32) for _ in range(3)]
    ln = [(0.032 * rng.standard_normal((J, C, G))).astype(np.float32) for _ in range(3)]
    o = kernel(xyz, tr, pl[0], pl[1], pl[2], ln[0], ln[1], ln[2])
    print(o.shape, o.dtype, float(o.max()))
